# revision 5
# baseline (speedup 1.0000x reference)
"""MoE gating kernel (logits -> softmax -> top-2 mask) for 8 trn2 NeuronCores.

Math: logits = x @ W.T + b  [B,S,E]; weights = softmax(logits, -1);
gated = weights masked to per-token top-2.  Returns (gated.T, weights.T),
both [E, B, S] fp32.

Strategy (v2):
  - Shard tokens (B*S = 65536) across 8 cores, 8192 tokens each.
  - fp32-class precision from fp16 splits:
        x ~= A + 2^-11 * B          (A, B fp16)
        W*2^8 ~= C + D' + 2^11*...  with C = fp16(W*2^8),
        D' = fp16((W - C*2^-8)*2^8), C'' = fp16(C*2^-11)
    so  logits*2^8 ~= A@C.T + A@D'.T + B@C''.T  -- ONE psum accumulator.
  - x must reach the PE transposed (d on partitions).  Hybrid transpose:
      * 5/16 of chunks ride the DMA xbar transpose (DRAM -> SBUF directly,
        ~157 GB/s effective),
      * 11/16 load straight (~358 GB/s) and are PE-transposed (fp16,
        1 cyc/row) with DVE/ACT evacuating PSUM -> SBUF.
    Split chosen so DMA-queue busy ~= PE busy ~= 125 us/core.
  - Per 1024-token group: 48 fp16 matmuls (N=512) accumulate logitsT*2^8
    [16,512] x2; evac to SBUF; PE transpose back to [128t, 8, 16];
    batched softmax: one exp (scale=2^-8) per group, segmented row-sums,
    reciprocal, per-tile max8 for the top-2 threshold, masked gate.
  - Outputs accumulate in SBUF as [(tile,e), group*t] via PE transpose and
    are written once at the end with one strided DMA per output.
  - Matmuls and transposes are interleaved per-chunk to keep the PE HAM
    clock warm (no >3.4us PE-idle windows).
"""

import functools

import numpy as np

NUM_CORES = 8
TOK_PER_CORE = 8192
GROUPS = 8  # groups per core
GTOK = 1024  # tokens per group
TILES = 8  # 128-token tiles per group
CHUNKS = 8  # d chunks of 128
D = 1024
E = 16

XS = 11  # x = A + 2^-XS * B
WS = 8  # logits scale: accumulating logits * 2^WS
XB_A = 3  # chunks of A via DMA-xbar transpose (0..XB_A-1)
XB_B = 2  # chunks of B via DMA-xbar transpose

TRACE = False
LAST_RESULTS = None


@functools.lru_cache(maxsize=2)
def _build(has_b: bool):
    from concourse import bacc, mybir
    import concourse.bass as bass
    import concourse.tile as tile
    from concourse.masks import make_identity

    f16 = mybir.dt.float16
    f32 = mybir.dt.float32
    Exp = mybir.ActivationFunctionType.Exp
    Op = mybir.AluOpType
    X = mybir.AxisListType.X

    nc = bacc.Bacc(
        "TRN2", target_bir_lowering=False, debug=False, num_devices=NUM_CORES
    )

    a_dram = nc.dram_tensor("a_t", [GROUPS, CHUNKS, GTOK, 128], f16, kind="ExternalInput").ap()
    b_dram = nc.dram_tensor("b_t", [GROUPS, CHUNKS, GTOK, 128], f16, kind="ExternalInput").ap()
    ct_dram = nc.dram_tensor("ct", [128, CHUNKS, E], f16, kind="ExternalInput").ap()
    dt_dram = nc.dram_tensor("dt", [128, CHUNKS, E], f16, kind="ExternalInput").ap()
    cs_dram = nc.dram_tensor("cs", [128, CHUNKS, E], f16, kind="ExternalInput").ap()
    if has_b:
        bcd_dram = nc.dram_tensor("bcd", [1, 2 * E], f16, kind="ExternalInput").ap()
    wts_dram = nc.dram_tensor("wts", [E, TOK_PER_CORE], f32, kind="ExternalOutput")
    gated_dram = nc.dram_tensor("gated", [E, TOK_PER_CORE], f32, kind="ExternalOutput")

    def bcast_inner(ap, n):
        return bass.AP(tensor=ap.tensor, offset=ap.offset, ap=[*ap.ap, [0, n]])

    with tile.TileContext(nc) as tc:
        with (
            tc.tile_pool(name="consts", bufs=1) as consts,
            tc.tile_pool(name="xt", bufs=2) as xt_pool,
            tc.tile_pool(name="nat", bufs=2) as nat_pool,
            tc.tile_pool(name="lg", bufs=2) as lg_pool,
            tc.tile_pool(name="sm", bufs=2) as sm_pool,
            tc.tile_pool(name="oacc", bufs=1) as oacc_pool,
            tc.tile_pool(name="pss", bufs=2, space="PSUM") as pss_pool,
            tc.tile_pool(name="pst", bufs=3, space="PSUM") as pst_pool,
            tc.tile_pool(name="pslgt", bufs=2, space="PSUM") as pslgt_pool,
            tc.tile_pool(name="psout", bufs=1, space="PSUM") as psout_pool,
        ):
            ct_sb = consts.tile([128, CHUNKS, E], f16)
            dt_sb = consts.tile([128, CHUNKS, E], f16)
            cs_sb = consts.tile([128, CHUNKS, E], f16)
            nc.sync.dma_start(out=ct_sb, in_=ct_dram)
            nc.sync.dma_start(out=dt_sb, in_=dt_dram)
            nc.sync.dma_start(out=cs_sb, in_=cs_dram)
            ident32 = consts.tile([128, 128], f32)
            make_identity(nc, ident32)
            ident16 = consts.tile([128, 128], f16)
            make_identity(nc, ident16)
            if has_b:
                bcd_sb = consts.tile([1, 2 * E], f16)
                nc.sync.dma_start(out=bcd_sb, in_=bcd_dram)
                ones_sb = consts.tile([1, 512], f16)
                nc.vector.memset(ones_sb, 1.0)

            # output accumulators: partition = (tile, e), free = (group, t)
            w_acc = oacc_pool.tile([128, GROUPS, 128], f32)
            g_acc = oacc_pool.tile([128, GROUPS, 128], f32)

            # straight-loaded chunk lists: (array_id, chunk)
            st_chunks = []
            for k in range(XB_B, CHUNKS):
                if k >= XB_A:
                    st_chunks.append((0, k))  # A
                st_chunks.append((1, k))  # B
            # order pairwise so mm(k) unblocks early: sort by chunk, B first
            st_chunks.sort(key=lambda t: (t[1], -t[0]))

            for g in range(GROUPS):
                xt_a = xt_pool.tile([128, CHUNKS, GTOK], f16, tag="xta")
                xt_b = xt_pool.tile([128, CHUNKS, GTOK], f16, tag="xtb")
                # xbar-transposed chunks straight into SBUF
                nc.sync.dma_start_transpose(
                    out=xt_a[:, 0:XB_A, :],
                    in_=a_dram[g, 0:XB_A].rearrange("k t d -> (k t) d"),
                )
                nc.sync.dma_start_transpose(
                    out=xt_b[:, 0:XB_B, :],
                    in_=b_dram[g, 0:XB_B].rearrange("k t d -> (k t) d"),
                )
                # straight loads of the rest (one DMA per array)
                nat_a = nat_pool.tile([128, CHUNKS - XB_A, TILES, 128], f16, tag="na")
                nat_b = nat_pool.tile([128, CHUNKS - XB_B, TILES, 128], f16, tag="nb")
                nc.scalar.dma_start(
                    out=nat_a,
                    in_=a_dram[g, XB_A:].rearrange("k (s p) d -> p k s d", p=128),
                )
                nc.scalar.dma_start(
                    out=nat_b,
                    in_=b_dram[g, XB_B:].rearrange("k (s p) d -> p k s d", p=128),
                )

                s_h = [
                    pss_pool.tile([E, 512], f32, tag="s", name=f"s_g{g}h{h}")
                    for h in range(2)
                ]

                def mms(k, last):
                    for h in range(2):
                        ra = xt_a[:, k, 512 * h : 512 * (h + 1)]
                        rb = xt_b[:, k, 512 * h : 512 * (h + 1)]
                        nc.tensor.matmul(
                            s_h[h], lhsT=ct_sb[:, k, :], rhs=ra,
                            start=(k == 0), stop=False,
                        )
                        nc.tensor.matmul(
                            s_h[h], lhsT=dt_sb[:, k, :], rhs=ra,
                            start=False, stop=False,
                        )
                        nc.tensor.matmul(
                            s_h[h], lhsT=cs_sb[:, k, :], rhs=rb,
                            start=False, stop=(last and not has_b),
                        )

                # interleave: xbar-covered matmuls first, then per straight
                # chunk transpose+evac with matmuls woven between
                mm_emitted = 0
                mms(0, False)
                mms(1, False)
                mm_emitted = 2
                evac_i = 0
                trans_done = {("A", k): False for _, k in st_chunks}
                ready_k = 2  # next k whose mms we may emit
                for idx, (arr, k) in enumerate(st_chunks):
                    nat, xt = (nat_a, xt_a) if arr == 0 else (nat_b, xt_b)
                    krel = k - (XB_A if arr == 0 else XB_B)
                    pst = pst_pool.tile([128, GTOK], f16, tag="pst")
                    for s in range(TILES):
                        nc.tensor.transpose(
                            pst[:, 128 * s : 128 * (s + 1)],
                            nat[:, krel, s, :],
                            ident16,
                        )
                    # evac psum -> sbuf (alternate DVE/ACT, DVE-heavy)
                    if evac_i % 3 == 2:
                        nc.scalar.copy(xt[:, k, :], pst)
                    else:
                        nc.vector.tensor_copy(xt[:, k, :], pst)
                    evac_i += 1
                    # after both arrays of chunk `ready_k` are evac'd, emit mms
                    while ready_k < CHUNKS:
                        a_ready = ready_k < XB_A or any(
                            (aa, kk) == (0, ready_k) for aa, kk in st_chunks[: idx + 1]
                        )
                        b_ready = ready_k < XB_B or any(
                            (aa, kk) == (1, ready_k) for aa, kk in st_chunks[: idx + 1]
                        )
                        if not (a_ready and b_ready):
                            break
                        mms(ready_k, ready_k == CHUNKS - 1)
                        ready_k += 1
                if has_b:
                    for h in range(2):
                        nc.tensor.matmul(
                            s_h[h], lhsT=bcd_sb[:, 0:E], rhs=ones_sb,
                            start=False, stop=False,
                        )
                        nc.tensor.matmul(
                            s_h[h], lhsT=bcd_sb[:, E : 2 * E], rhs=ones_sb,
                            start=False, stop=True,
                        )
                assert ready_k == CHUNKS

                # scaled logits.T for the group -> SBUF
                lgS = lg_pool.tile([E, GTOK], f32)
                for h in range(2):
                    nc.scalar.copy(lgS[:, 512 * h : 512 * (h + 1)], s_h[h])

                # transpose back to [128 t, 8, 16]
                lgt_ps = pslgt_pool.tile([128, TILES, E], f32)
                for i in range(TILES):
                    nc.tensor.transpose(
                        lgt_ps[:, i, :],
                        lgS[:, 128 * i : 128 * (i + 1)],
                        ident32[:E, :E],
                    )
                lgt = sm_pool.tile([128, TILES, E], f32, tag="lgt")
                nc.vector.tensor_copy(lgt, lgt_ps)

                # batched softmax + top-2 gate
                m8 = sm_pool.tile([128, TILES, 8], f32, tag="m8")
                for i in range(TILES):
                    nc.vector.max(m8[:, i, :], lgt[:, i, :])
                ex = sm_pool.tile([128, TILES, E], f32, tag="ex")
                nc.scalar.activation(ex, lgt, func=Exp, scale=float(2.0**-WS))
                ssum = sm_pool.tile([128, TILES], f32, tag="ssum")
                nc.vector.tensor_reduce(ssum, ex, axis=X, op=Op.add)
                rec = sm_pool.tile([128, TILES], f32, tag="rec")
                nc.vector.reciprocal(rec, ssum)
                w_grp = sm_pool.tile([128, TILES, E], f32, tag="wg")
                nc.vector.tensor_tensor(
                    out=w_grp, in0=ex, in1=bcast_inner(rec[:, :], E), op=Op.mult
                )
                msk = sm_pool.tile([128, TILES, E], f32, tag="msk")
                nc.vector.tensor_tensor(
                    out=msk, in0=lgt, in1=bcast_inner(m8[:, :, 1], E), op=Op.is_ge
                )
                g_grp = sm_pool.tile([128, TILES, E], f32, tag="gg")
                nc.vector.tensor_tensor(out=g_grp, in0=msk, in1=w_grp, op=Op.mult)

                # transpose outputs to [(tile,e), t] and stash
                ps_o = psout_pool.tile([128, 256], f32)
                nc.tensor.transpose(ps_o[:, 0:128], w_grp, ident32)
                nc.tensor.transpose(ps_o[:, 128:256], g_grp, ident32)
                nc.scalar.copy(w_acc[:, g, :], ps_o[:, 0:128])
                nc.vector.tensor_copy(g_acc[:, g, :], ps_o[:, 128:256])

            # final writeback: partition p=(tile,e); addr = e*8192+g*1024+tile*128+t
            out_ap = [[128, TILES], [TOK_PER_CORE, E], [GTOK, GROUPS], [1, 128]]
            nc.sync.dma_start(
                out=bass.AP(tensor=wts_dram, offset=0, ap=list(out_ap)), in_=w_acc
            )
            nc.sync.dma_start(
                out=bass.AP(tensor=gated_dram, offset=0, ap=list(out_ap)), in_=g_acc
            )

    nc.compile()
    return nc


def _split_x(xf):
    a = xf.astype(np.float16)
    b = ((xf - a.astype(np.float32)) * np.float32(2.0**XS)).astype(np.float16)
    return a, b


def _w_consts(W):
    C = (W * np.float32(2.0**WS)).astype(np.float16)
    Dp = ((W - C.astype(np.float32) * np.float32(2.0**-WS)) * np.float32(2.0**WS)).astype(np.float16)
    Cs = (C.astype(np.float32) * np.float32(2.0**-XS)).astype(np.float16)

    def lay(M):  # [16, 1024] -> [128 d_lo, chunks, E]
        return np.ascontiguousarray(M.T.reshape(CHUNKS, 128, E).transpose(1, 0, 2))

    return lay(C), lay(Dp), lay(Cs)


def kernel(x, W, b):
    global LAST_RESULTS
    from concourse.bass_utils import run_bass_kernel_spmd

    x = np.ascontiguousarray(np.asarray(x, dtype=np.float32))
    W = np.ascontiguousarray(np.asarray(W, dtype=np.float32))
    b = np.ascontiguousarray(np.asarray(b, dtype=np.float32))
    Bb, S, Dd = x.shape
    ntok = Bb * S
    assert (ntok, Dd) == (NUM_CORES * TOK_PER_CORE, D) and W.shape == (E, D)

    xf = x.reshape(ntok, D)
    A, Bx = _split_x(xf)

    def blocked(arr, c):
        sh = arr[c * TOK_PER_CORE : (c + 1) * TOK_PER_CORE]
        return np.ascontiguousarray(
            sh.reshape(GROUPS, GTOK, CHUNKS, 128).transpose(0, 2, 1, 3)
        )

    ct, dt, cs = _w_consts(W)

    has_b = bool(np.any(b))
    in_maps = []
    for c in range(NUM_CORES):
        m = {"a_t": blocked(A, c), "b_t": blocked(Bx, c), "ct": ct, "dt": dt, "cs": cs}
        if has_b:
            bc = (b * np.float32(2.0**WS)).astype(np.float16)
            bd = ((b - bc.astype(np.float32) * np.float32(2.0**-WS)) * np.float32(2.0**WS)).astype(np.float16)
            m["bcd"] = np.concatenate([bc, bd]).reshape(1, 2 * E)
        in_maps.append(m)

    nc = _build(has_b)
    res = run_bass_kernel_spmd(
        nc, in_maps, core_ids=list(range(NUM_CORES)), trace=TRACE
    )
    LAST_RESULTS = res

    wts = np.concatenate([r["wts"] for r in res.results], axis=1)
    gated = np.concatenate([r["gated"] for r in res.results], axis=1)
    return (
        gated.reshape(E, Bb, S).astype(np.float32),
        wts.reshape(E, Bb, S).astype(np.float32),
    )


# revision 6
# speedup vs baseline: 1.0641x; 1.0641x over previous
"""MoE gating kernel (logits -> softmax -> top-2 mask) for 8 trn2 NeuronCores.

Math: logits = x @ W.T + b  [B,S,E]; weights = softmax(logits, -1);
gated = weights masked to per-token top-2.  Returns (gated.T, weights.T),
both [E, B, S] fp32.

Strategy (v3):
  - Shard tokens (B*S = 65536) across 8 cores, 8192 tokens each.
  - fp32-class precision from fp16 splits with power-of-2 scales:
        x ~= A + 2^-11 * B                    (A, B fp16)
        logits*2^8 ~= A@C.T + A@D'.T + B@C''.T    (one PSUM accumulator)
    where C = fp16(W*2^8), D' = fp16((W - C*2^-8)*2^8), C'' = fp16(C*2^-11).
  - x must reach the PE with d on partitions.  Hybrid transpose:
      * XB_A + XB_B chunks ride the DMA xbar transpose from a host-blocked
        [1024,128]-contiguous layout (measured ~171 GB/s effective),
      * remaining chunks load straight from the unblocked row-major layout
        (1-2 KB contiguous rows, ~358 GB/s) and are PE-transposed with
        DVE/ACT evacuating PSUM -> SBUF.
    PE transposes cost ~250ns/[128,128] (stationary reload), so only a
    small fraction goes through the PE; split tuned so DMA-pool busy ~= PE
    busy.
  - Per 1024-token group: 48 fp16 matmuls (N=512) accumulate logitsT*2^8;
    PE transposes interleave between matmul bursts to keep HAM warm.
  - Batched softmax per group: one exp(scale=2^-8), segmented row-sums,
    reciprocal, per-tile max8 threshold, fused top-2 gate.
  - Outputs accumulate in SBUF as [(tile,e), (group,t)] via PE transpose,
    written once at the end with one strided DMA per output.
"""

import functools

import numpy as np

NUM_CORES = 8
TOK_PER_CORE = 8192
GROUPS = 8
GTOK = 1024
TILES = 8
CHUNKS = 8
D = 1024
E = 16

XS = 11  # x = A + 2^-XS * B
WS = 8  # accumulating logits * 2^WS
XB_A = 6  # chunks of A via DMA-xbar transpose (0..XB_A-1)
XB_B = 6  # chunks of B via DMA-xbar transpose

TRACE = False
LAST_RESULTS = None


@functools.lru_cache(maxsize=2)
def _build(has_b: bool):
    from concourse import bacc, mybir
    import concourse.bass as bass
    import concourse.tile as tile
    from concourse.masks import make_identity

    f16 = mybir.dt.float16
    f32 = mybir.dt.float32
    Exp = mybir.ActivationFunctionType.Exp
    Op = mybir.AluOpType
    X = mybir.AxisListType.X

    ST_A = CHUNKS - XB_A  # straight chunks of A
    ST_B = CHUNKS - XB_B

    nc = bacc.Bacc(
        "TRN2", target_bir_lowering=False, debug=False, num_devices=NUM_CORES
    )

    axb_dram = nc.dram_tensor("a_xb", [GROUPS, XB_A, GTOK, 128], f16, kind="ExternalInput").ap()
    bxb_dram = nc.dram_tensor("b_xb", [GROUPS, XB_B, GTOK, 128], f16, kind="ExternalInput").ap()
    ast_dram = nc.dram_tensor("a_st", [TOK_PER_CORE, ST_A * 128], f16, kind="ExternalInput").ap()
    bst_dram = nc.dram_tensor("b_st", [TOK_PER_CORE, ST_B * 128], f16, kind="ExternalInput").ap()
    ct_dram = nc.dram_tensor("ct", [128, CHUNKS, E], f16, kind="ExternalInput").ap()
    dt_dram = nc.dram_tensor("dt", [128, CHUNKS, E], f16, kind="ExternalInput").ap()
    cs_dram = nc.dram_tensor("cs", [128, CHUNKS, E], f16, kind="ExternalInput").ap()
    if has_b:
        bcd_dram = nc.dram_tensor("bcd", [1, 2 * E], f16, kind="ExternalInput").ap()
    wts_dram = nc.dram_tensor("wts", [E, TOK_PER_CORE], f32, kind="ExternalOutput")
    gated_dram = nc.dram_tensor("gated", [E, TOK_PER_CORE], f32, kind="ExternalOutput")

    def bcast_inner(ap, n):
        return bass.AP(tensor=ap.tensor, offset=ap.offset, ap=[*ap.ap, [0, n]])

    with tile.TileContext(nc) as tc:
        with (
            tc.tile_pool(name="consts", bufs=1) as consts,
            tc.tile_pool(name="xt", bufs=2) as xt_pool,
            tc.tile_pool(name="nat", bufs=2) as nat_pool,
            tc.tile_pool(name="lg", bufs=2) as lg_pool,
            tc.tile_pool(name="sm", bufs=2) as sm_pool,
            tc.tile_pool(name="oacc", bufs=1) as oacc_pool,
            tc.tile_pool(name="pss", bufs=4, space="PSUM") as pss_pool,
            tc.tile_pool(name="pst", bufs=2, space="PSUM") as pst_pool,
            tc.tile_pool(name="pslgt", bufs=1, space="PSUM") as pslgt_pool,
            tc.tile_pool(name="psout", bufs=1, space="PSUM") as psout_pool,
        ):
            ct_sb = consts.tile([128, CHUNKS, E], f16)
            dt_sb = consts.tile([128, CHUNKS, E], f16)
            cs_sb = consts.tile([128, CHUNKS, E], f16)
            nc.sync.dma_start(out=ct_sb, in_=ct_dram)
            nc.sync.dma_start(out=dt_sb, in_=dt_dram)
            nc.sync.dma_start(out=cs_sb, in_=cs_dram)
            ident32 = consts.tile([128, 128], f32)
            make_identity(nc, ident32)
            ident16 = consts.tile([128, 128], f16)
            make_identity(nc, ident16)
            if has_b:
                bcd_sb = consts.tile([1, 2 * E], f16)
                nc.sync.dma_start(out=bcd_sb, in_=bcd_dram)
                ones_sb = consts.tile([1, 512], f16)
                nc.vector.memset(ones_sb, 1.0)

            w_acc = oacc_pool.tile([128, GROUPS, 128], f32)
            g_acc = oacc_pool.tile([128, GROUPS, 128], f32)

            # straight-chunk order: alternate arrays for even interleave
            st_chunks = []
            for i in range(max(ST_A, ST_B)):
                if i < ST_A:
                    st_chunks.append((0, XB_A + i))
                if i < ST_B:
                    st_chunks.append((1, XB_B + i))

            for g in range(GROUPS):
                xt_a = xt_pool.tile([128, CHUNKS, GTOK], f16, tag="xta")
                xt_b = xt_pool.tile([128, CHUNKS, GTOK], f16, tag="xtb")
                nc.sync.dma_start_transpose(
                    out=xt_a[:, 0:XB_A, :],
                    in_=axb_dram[g].rearrange("k t d -> (k t) d"),
                )
                nc.sync.dma_start_transpose(
                    out=xt_b[:, 0:XB_B, :],
                    in_=bxb_dram[g].rearrange("k t d -> (k t) d"),
                )
                nat_a = nat_pool.tile([128, TILES, ST_A * 128], f16, tag="na")
                nat_b = nat_pool.tile([128, TILES, ST_B * 128], f16, tag="nb")
                nc.scalar.dma_start(
                    out=nat_a,
                    in_=ast_dram[g * GTOK : (g + 1) * GTOK].rearrange(
                        "(s p) d -> p s d", p=128
                    ),
                )
                nc.scalar.dma_start(
                    out=nat_b,
                    in_=bst_dram[g * GTOK : (g + 1) * GTOK].rearrange(
                        "(s p) d -> p s d", p=128
                    ),
                )

                s_h = [
                    pss_pool.tile([E, 512], f32, tag="s", name=f"s_g{g}h{h}")
                    for h in range(2)
                ]

                def mms(k, last):
                    for h in range(2):
                        ra = xt_a[:, k, 512 * h : 512 * (h + 1)]
                        rb = xt_b[:, k, 512 * h : 512 * (h + 1)]
                        nc.tensor.matmul(
                            s_h[h], lhsT=ct_sb[:, k, :], rhs=ra,
                            start=(k == 0), stop=False,
                        )
                        nc.tensor.matmul(
                            s_h[h], lhsT=dt_sb[:, k, :], rhs=ra,
                            start=False, stop=False,
                        )
                        nc.tensor.matmul(
                            s_h[h], lhsT=cs_sb[:, k, :], rhs=rb,
                            start=False, stop=(last and not has_b),
                        )

                # emit: one xbar-chunk mm burst, then one straight-chunk
                # transpose+evac, alternating; straight mms at the end
                n_xb = min(XB_A, XB_B)
                for i, (arr, k) in enumerate(st_chunks):
                    if i < n_xb:
                        mms(i, False)
                    nat, xt, st0 = (
                        (nat_a, xt_a, XB_A) if arr == 0 else (nat_b, xt_b, XB_B)
                    )
                    krel = k - st0
                    pst = pst_pool.tile([128, GTOK], f16, tag="pst")
                    for s in range(TILES):
                        nc.tensor.transpose(
                            pst[:, 128 * s : 128 * (s + 1)],
                            nat[:, s, 128 * krel : 128 * (krel + 1)],
                            ident16,
                        )
                    if i % 2 == 0:
                        nc.vector.tensor_copy(xt[:, k, :], pst)
                    else:
                        nc.scalar.copy(xt[:, k, :], pst)
                for k in range(len(st_chunks), n_xb):
                    mms(k, False)
                for k in range(n_xb, CHUNKS):
                    mms(k, k == CHUNKS - 1)
                if has_b:
                    for h in range(2):
                        nc.tensor.matmul(
                            s_h[h], lhsT=bcd_sb[:, 0:E], rhs=ones_sb,
                            start=False, stop=False,
                        )
                        nc.tensor.matmul(
                            s_h[h], lhsT=bcd_sb[:, E : 2 * E], rhs=ones_sb,
                            start=False, stop=True,
                        )

                lgS = lg_pool.tile([E, GTOK], f32)
                for h in range(2):
                    nc.scalar.copy(lgS[:, 512 * h : 512 * (h + 1)], s_h[h])

                lgt_ps = pslgt_pool.tile([128, TILES, E], f32)
                for i in range(TILES):
                    nc.tensor.transpose(
                        lgt_ps[:, i, :],
                        lgS[:, 128 * i : 128 * (i + 1)],
                        ident32[:E, :E],
                    )
                lgt = sm_pool.tile([128, TILES, E], f32, tag="lgt")
                nc.vector.tensor_copy(lgt, lgt_ps)

                m8 = sm_pool.tile([128, TILES, 8], f32, tag="m8")
                for i in range(TILES):
                    nc.vector.max(m8[:, i, :], lgt[:, i, :])
                ex = sm_pool.tile([128, TILES, E], f32, tag="ex")
                nc.scalar.activation(ex, lgt, func=Exp, scale=float(2.0**-WS))
                ssum = sm_pool.tile([128, TILES], f32, tag="ssum")
                nc.vector.tensor_reduce(ssum, ex, axis=X, op=Op.add)
                rec = sm_pool.tile([128, TILES], f32, tag="rec")
                nc.vector.reciprocal(rec, ssum)
                w_grp = sm_pool.tile([128, TILES, E], f32, tag="wg")
                nc.vector.tensor_tensor(
                    out=w_grp, in0=ex, in1=bcast_inner(rec[:, :], E), op=Op.mult
                )
                msk = sm_pool.tile([128, TILES, E], f32, tag="msk")
                nc.vector.tensor_tensor(
                    out=msk, in0=lgt, in1=bcast_inner(m8[:, :, 1], E), op=Op.is_ge
                )
                g_grp = sm_pool.tile([128, TILES, E], f32, tag="gg")
                nc.vector.tensor_tensor(out=g_grp, in0=msk, in1=w_grp, op=Op.mult)

                ps_o = psout_pool.tile([128, 256], f32)
                nc.tensor.transpose(ps_o[:, 0:128], w_grp, ident32)
                nc.tensor.transpose(ps_o[:, 128:256], g_grp, ident32)
                nc.scalar.copy(w_acc[:, g, :], ps_o[:, 0:128])
                nc.vector.tensor_copy(g_acc[:, g, :], ps_o[:, 128:256])

            out_ap = [[128, TILES], [TOK_PER_CORE, E], [GTOK, GROUPS], [1, 128]]
            nc.sync.dma_start(
                out=bass.AP(tensor=wts_dram, offset=0, ap=list(out_ap)), in_=w_acc
            )
            nc.sync.dma_start(
                out=bass.AP(tensor=gated_dram, offset=0, ap=list(out_ap)), in_=g_acc
            )

    nc.compile()
    return nc


def _split_x(xf):
    a = xf.astype(np.float16)
    b = ((xf - a.astype(np.float32)) * np.float32(2.0**XS)).astype(np.float16)
    return a, b


def _w_consts(W):
    C = (W * np.float32(2.0**WS)).astype(np.float16)
    Dp = ((W - C.astype(np.float32) * np.float32(2.0**-WS)) * np.float32(2.0**WS)).astype(np.float16)
    Cs = (C.astype(np.float32) * np.float32(2.0**-XS)).astype(np.float16)

    def lay(M):  # [16, 1024] -> [128 d_lo, chunks, E]
        return np.ascontiguousarray(M.T.reshape(CHUNKS, 128, E).transpose(1, 0, 2))

    return lay(C), lay(Dp), lay(Cs)


def kernel(x, W, b):
    global LAST_RESULTS
    from concourse.bass_utils import run_bass_kernel_spmd

    x = np.ascontiguousarray(np.asarray(x, dtype=np.float32))
    W = np.ascontiguousarray(np.asarray(W, dtype=np.float32))
    b = np.ascontiguousarray(np.asarray(b, dtype=np.float32))
    Bb, S, Dd = x.shape
    ntok = Bb * S
    assert (ntok, Dd) == (NUM_CORES * TOK_PER_CORE, D) and W.shape == (E, D)

    xf = x.reshape(ntok, D)
    A, Bx = _split_x(xf)

    def xb_blocked(arr, c, nxb):
        sh = arr[c * TOK_PER_CORE : (c + 1) * TOK_PER_CORE, : nxb * 128]
        return np.ascontiguousarray(
            sh.reshape(GROUPS, GTOK, nxb, 128).transpose(0, 2, 1, 3)
        )

    def straight(arr, c, nxb):
        return np.ascontiguousarray(arr[c * TOK_PER_CORE : (c + 1) * TOK_PER_CORE, nxb * 128 :])

    ct, dt, cs = _w_consts(W)

    has_b = bool(np.any(b))
    in_maps = []
    for c in range(NUM_CORES):
        m = {
            "a_xb": xb_blocked(A, c, XB_A),
            "b_xb": xb_blocked(Bx, c, XB_B),
            "a_st": straight(A, c, XB_A),
            "b_st": straight(Bx, c, XB_B),
            "ct": ct,
            "dt": dt,
            "cs": cs,
        }
        if has_b:
            bc = (b * np.float32(2.0**WS)).astype(np.float16)
            bd = ((b - bc.astype(np.float32) * np.float32(2.0**-WS)) * np.float32(2.0**WS)).astype(np.float16)
            m["bcd"] = np.concatenate([bc, bd]).reshape(1, 2 * E)
        in_maps.append(m)

    nc = _build(has_b)
    res = run_bass_kernel_spmd(
        nc, in_maps, core_ids=list(range(NUM_CORES)), trace=TRACE
    )
    LAST_RESULTS = res

    wts = np.concatenate([r["wts"] for r in res.results], axis=1)
    gated = np.concatenate([r["gated"] for r in res.results], axis=1)
    return (
        gated.reshape(E, Bb, S).astype(np.float32),
        wts.reshape(E, Bb, S).astype(np.float32),
    )


# revision 7
# speedup vs baseline: 1.0774x; 1.0125x over previous
"""MoE gating kernel (logits -> softmax -> top-2 mask) for 8 trn2 NeuronCores.

Math: logits = x @ W.T + b  [B,S,E]; weights = softmax(logits, -1);
gated = weights masked to per-token top-2.  Returns (gated.T, weights.T),
both [E, B, S] fp32.

Strategy (v3):
  - Shard tokens (B*S = 65536) across 8 cores, 8192 tokens each.
  - fp32-class precision from fp16 splits with power-of-2 scales:
        x ~= A + 2^-11 * B                    (A, B fp16)
        logits*2^8 ~= A@C.T + A@D'.T + B@C''.T    (one PSUM accumulator)
    where C = fp16(W*2^8), D' = fp16((W - C*2^-8)*2^8), C'' = fp16(C*2^-11).
  - x must reach the PE with d on partitions.  Hybrid transpose:
      * XB_A + XB_B chunks ride the DMA xbar transpose from a host-blocked
        [1024,128]-contiguous layout (measured ~171 GB/s effective),
      * remaining chunks load straight from the unblocked row-major layout
        (1-2 KB contiguous rows, ~358 GB/s) and are PE-transposed with
        DVE/ACT evacuating PSUM -> SBUF.
    PE transposes cost ~250ns/[128,128] (stationary reload), so only a
    small fraction goes through the PE; split tuned so DMA-pool busy ~= PE
    busy.
  - Per 1024-token group: 48 fp16 matmuls (N=512) accumulate logitsT*2^8;
    PE transposes interleave between matmul bursts to keep HAM warm.
  - Batched softmax per group: one exp(scale=2^-8), segmented row-sums,
    reciprocal, per-tile max8 threshold, fused top-2 gate.
  - Outputs accumulate in SBUF as [(tile,e), (group,t)] via PE transpose,
    written once at the end with one strided DMA per output.
"""

import functools

import numpy as np

NUM_CORES = 8
TOK_PER_CORE = 8192
GROUPS = 8
GTOK = 1024
TILES = 8
CHUNKS = 8
D = 1024
E = 16

XS = 11  # x = A + 2^-XS * B
WS = 8  # accumulating logits * 2^WS
XB_A = 6  # chunks of A via DMA-xbar transpose (0..XB_A-1)
XB_B = 6  # chunks of B via DMA-xbar transpose

TRACE = False
LAST_RESULTS = None


@functools.lru_cache(maxsize=2)
def _build(has_b: bool):
    from concourse import bacc, mybir
    import concourse.bass as bass
    import concourse.tile as tile
    from concourse.masks import make_identity

    f16 = mybir.dt.float16
    f32 = mybir.dt.float32
    Exp = mybir.ActivationFunctionType.Exp
    Op = mybir.AluOpType
    X = mybir.AxisListType.X

    ST_A = CHUNKS - XB_A  # straight chunks of A
    ST_B = CHUNKS - XB_B

    nc = bacc.Bacc(
        "TRN2", target_bir_lowering=False, debug=False, num_devices=NUM_CORES
    )

    axb_dram = nc.dram_tensor("a_xb", [GROUPS, XB_A, GTOK, 128], f16, kind="ExternalInput").ap()
    bxb_dram = nc.dram_tensor("b_xb", [GROUPS, XB_B, GTOK, 128], f16, kind="ExternalInput").ap()
    ast_dram = nc.dram_tensor("a_st", [TOK_PER_CORE, ST_A * 128], f16, kind="ExternalInput").ap()
    bst_dram = nc.dram_tensor("b_st", [TOK_PER_CORE, ST_B * 128], f16, kind="ExternalInput").ap()
    cda_dram = nc.dram_tensor("cda", [128, CHUNKS, 4 * E], f16, kind="ExternalInput").ap()
    cs_dram = nc.dram_tensor("cs", [128, CHUNKS, E], f16, kind="ExternalInput").ap()
    if has_b:
        bcd_dram = nc.dram_tensor("bcd", [1, 4 * E], f16, kind="ExternalInput").ap()
    wts_dram = nc.dram_tensor("wts", [E, TOK_PER_CORE], f32, kind="ExternalOutput")
    gated_dram = nc.dram_tensor("gated", [E, TOK_PER_CORE], f32, kind="ExternalOutput")

    def bcast_inner(ap, n):
        return bass.AP(tensor=ap.tensor, offset=ap.offset, ap=[*ap.ap, [0, n]])

    with tile.TileContext(nc) as tc:
        with (
            tc.tile_pool(name="consts", bufs=1) as consts,
            tc.tile_pool(name="xt", bufs=2) as xt_pool,
            tc.tile_pool(name="nat", bufs=2) as nat_pool,
            tc.tile_pool(name="lg", bufs=2) as lg_pool,
            tc.tile_pool(name="sm", bufs=2) as sm_pool,
            tc.tile_pool(name="oacc", bufs=1) as oacc_pool,
            tc.tile_pool(name="pss", bufs=4, space="PSUM") as pss_pool,
            tc.tile_pool(name="pst", bufs=2, space="PSUM") as pst_pool,
            tc.tile_pool(name="pslgt", bufs=1, space="PSUM") as pslgt_pool,
            tc.tile_pool(name="psout", bufs=1, space="PSUM") as psout_pool,
        ):
            cda_sb = consts.tile([128, CHUNKS, 4 * E], f16)
            cs_sb = consts.tile([128, CHUNKS, E], f16)
            nc.sync.dma_start(out=cda_sb, in_=cda_dram)
            nc.sync.dma_start(out=cs_sb, in_=cs_dram)
            ident32 = consts.tile([128, 128], f32)
            make_identity(nc, ident32)
            ident16 = consts.tile([128, 128], f16)
            make_identity(nc, ident16)
            if has_b:
                bcd_sb = consts.tile([1, 4 * E], f16)
                nc.sync.dma_start(out=bcd_sb, in_=bcd_dram)
                ones_sb = consts.tile([1, 512], f16)
                nc.vector.memset(ones_sb, 1.0)
                zeros_sb = consts.tile([1, 512], f16)
                nc.vector.memset(zeros_sb, 0.0)

            w_acc = oacc_pool.tile([128, GROUPS, 128], f32)
            g_acc = oacc_pool.tile([128, GROUPS, 128], f32)

            # straight-chunk order: alternate arrays for even interleave
            st_chunks = []
            for i in range(max(ST_A, ST_B)):
                if i < ST_A:
                    st_chunks.append((0, XB_A + i))
                if i < ST_B:
                    st_chunks.append((1, XB_B + i))

            for g in range(GROUPS):
                xt_a = xt_pool.tile([128, CHUNKS, GTOK], f16, tag="xta")
                xt_b = xt_pool.tile([128, CHUNKS, GTOK], f16, tag="xtb")
                nc.sync.dma_start_transpose(
                    out=xt_a[:, 0:XB_A, :],
                    in_=axb_dram[g].rearrange("k t d -> (k t) d"),
                )
                nc.sync.dma_start_transpose(
                    out=xt_b[:, 0:XB_B, :],
                    in_=bxb_dram[g].rearrange("k t d -> (k t) d"),
                )
                nat_a = nat_pool.tile([128, TILES, ST_A * 128], f16, tag="na")
                nat_b = nat_pool.tile([128, TILES, ST_B * 128], f16, tag="nb")
                nc.scalar.dma_start(
                    out=nat_a,
                    in_=ast_dram[g * GTOK : (g + 1) * GTOK].rearrange(
                        "(s p) d -> p s d", p=128
                    ),
                )
                nc.scalar.dma_start(
                    out=nat_b,
                    in_=bst_dram[g * GTOK : (g + 1) * GTOK].rearrange(
                        "(s p) d -> p s d", p=128
                    ),
                )

                s_h = [
                    pss_pool.tile([128, 512], f32, tag="s", name=f"s_g{g}h{h}")
                    for h in range(2)
                ]

                def mms(k, last):
                    for h in range(2):
                        ra = xt_a[:, k, 512 * h : 512 * (h + 1)]
                        rb = xt_b[:, k, 512 * h : 512 * (h + 1)]
                        nc.tensor.matmul(
                            s_h[h][0:64, :], lhsT=cda_sb[:, k, :], rhs=ra,
                            start=(k == 0), stop=(last and not has_b),
                            tile_position=(0, 0),
                        )
                        nc.tensor.matmul(
                            s_h[h][64:80, :], lhsT=cs_sb[:, k, :], rhs=rb,
                            start=(k == 0), stop=(last and not has_b),
                            tile_position=(0, 64),
                        )

                # emit: one xbar-chunk mm burst, then one straight-chunk
                # transpose+evac, alternating; straight mms at the end
                n_xb = min(XB_A, XB_B)
                for i, (arr, k) in enumerate(st_chunks):
                    if i < n_xb:
                        mms(i, False)
                    nat, xt, st0 = (
                        (nat_a, xt_a, XB_A) if arr == 0 else (nat_b, xt_b, XB_B)
                    )
                    krel = k - st0
                    pst = pst_pool.tile([128, GTOK], f16, tag="pst")
                    for s in range(TILES):
                        nc.tensor.transpose(
                            pst[:, 128 * s : 128 * (s + 1)],
                            nat[:, s, 128 * krel : 128 * (krel + 1)],
                            ident16,
                        )
                    if i % 2 == 0:
                        nc.vector.tensor_copy(xt[:, k, :], pst)
                    else:
                        nc.scalar.copy(xt[:, k, :], pst)
                for k in range(len(st_chunks), n_xb):
                    mms(k, False)
                for k in range(n_xb, CHUNKS):
                    mms(k, k == CHUNKS - 1)
                if has_b:
                    for h in range(2):
                        nc.tensor.matmul(
                            s_h[h][0:64, :], lhsT=bcd_sb, rhs=ones_sb,
                            start=False, stop=True, tile_position=(0, 0),
                        )
                        nc.tensor.matmul(
                            s_h[h][64:80, :], lhsT=cs_sb[:, 0, :],
                            rhs=zeros_sb, start=False, stop=True,
                            tile_position=(0, 64),
                        )

                lgS = lg_pool.tile([E, GTOK], f32)
                for h in range(2):
                    cmb = sm_pool.tile([E, 512], f32, tag="cmb")
                    nc.scalar.copy(cmb, s_h[h][0:16, :])
                    nc.vector.tensor_add(cmb, cmb, s_h[h][32:48, :])
                    nc.vector.tensor_add(
                        lgS[:, 512 * h : 512 * (h + 1)], cmb, s_h[h][64:80, :]
                    )

                lgt_ps = pslgt_pool.tile([128, TILES, E], f32)
                for i in range(TILES):
                    nc.tensor.transpose(
                        lgt_ps[:, i, :],
                        lgS[:, 128 * i : 128 * (i + 1)],
                        ident32[:E, :E],
                    )
                lgt = sm_pool.tile([128, TILES, E], f32, tag="lgt")
                nc.vector.tensor_copy(lgt, lgt_ps)

                m8 = sm_pool.tile([128, TILES, 8], f32, tag="m8")
                for i in range(TILES):
                    nc.vector.max(m8[:, i, :], lgt[:, i, :])
                ex = sm_pool.tile([128, TILES, E], f32, tag="ex")
                nc.scalar.activation(ex, lgt, func=Exp, scale=float(2.0**-WS))
                ssum = sm_pool.tile([128, TILES], f32, tag="ssum")
                nc.vector.tensor_reduce(ssum, ex, axis=X, op=Op.add)
                rec = sm_pool.tile([128, TILES], f32, tag="rec")
                nc.vector.reciprocal(rec, ssum)
                w_grp = sm_pool.tile([128, TILES, E], f32, tag="wg")
                nc.vector.tensor_tensor(
                    out=w_grp, in0=ex, in1=bcast_inner(rec[:, :], E), op=Op.mult
                )
                msk = sm_pool.tile([128, TILES, E], f32, tag="msk")
                nc.vector.tensor_tensor(
                    out=msk, in0=lgt, in1=bcast_inner(m8[:, :, 1], E), op=Op.is_ge
                )
                g_grp = sm_pool.tile([128, TILES, E], f32, tag="gg")
                nc.vector.tensor_tensor(out=g_grp, in0=msk, in1=w_grp, op=Op.mult)

                ps_o = psout_pool.tile([128, 256], f32)
                nc.tensor.transpose(ps_o[:, 0:128], w_grp, ident32)
                nc.tensor.transpose(ps_o[:, 128:256], g_grp, ident32)
                nc.scalar.copy(w_acc[:, g, :], ps_o[:, 0:128])
                nc.vector.tensor_copy(g_acc[:, g, :], ps_o[:, 128:256])

            out_ap = [[128, TILES], [TOK_PER_CORE, E], [GTOK, GROUPS], [1, 128]]
            nc.sync.dma_start(
                out=bass.AP(tensor=wts_dram, offset=0, ap=list(out_ap)), in_=w_acc
            )
            nc.sync.dma_start(
                out=bass.AP(tensor=gated_dram, offset=0, ap=list(out_ap)), in_=g_acc
            )

    nc.compile()
    return nc


def _split_x(xf):
    a = xf.astype(np.float16)
    b = ((xf - a.astype(np.float32)) * np.float32(2.0**XS)).astype(np.float16)
    return a, b


def _w_consts(W):
    C = (W * np.float32(2.0**WS)).astype(np.float16)
    Dp = ((W - C.astype(np.float32) * np.float32(2.0**-WS)) * np.float32(2.0**WS)).astype(np.float16)
    Cs = (C.astype(np.float32) * np.float32(2.0**-XS)).astype(np.float16)

    def lay(M):  # [16, 1024] -> [128 d_lo, chunks, E]
        return np.ascontiguousarray(M.T.reshape(CHUNKS, 128, E).transpose(1, 0, 2))

    # packed A-side stationary: [C | 0 | D' | 0] -> [128, chunks, 64]
    cda = np.zeros((128, CHUNKS, 4 * E), np.float16)
    cda[:, :, 0:E] = lay(C)
    cda[:, :, 2 * E : 3 * E] = lay(Dp)
    return cda, lay(Cs)


def kernel(x, W, b):
    global LAST_RESULTS
    from concourse.bass_utils import run_bass_kernel_spmd

    x = np.ascontiguousarray(np.asarray(x, dtype=np.float32))
    W = np.ascontiguousarray(np.asarray(W, dtype=np.float32))
    b = np.ascontiguousarray(np.asarray(b, dtype=np.float32))
    Bb, S, Dd = x.shape
    ntok = Bb * S
    assert (ntok, Dd) == (NUM_CORES * TOK_PER_CORE, D) and W.shape == (E, D)

    xf = x.reshape(ntok, D)
    A, Bx = _split_x(xf)

    def xb_blocked(arr, c, nxb):
        sh = arr[c * TOK_PER_CORE : (c + 1) * TOK_PER_CORE, : nxb * 128]
        return np.ascontiguousarray(
            sh.reshape(GROUPS, GTOK, nxb, 128).transpose(0, 2, 1, 3)
        )

    def straight(arr, c, nxb):
        return np.ascontiguousarray(arr[c * TOK_PER_CORE : (c + 1) * TOK_PER_CORE, nxb * 128 :])

    cda, cs = _w_consts(W)

    has_b = bool(np.any(b))
    in_maps = []
    for c in range(NUM_CORES):
        m = {
            "a_xb": xb_blocked(A, c, XB_A),
            "b_xb": xb_blocked(Bx, c, XB_B),
            "a_st": straight(A, c, XB_A),
            "b_st": straight(Bx, c, XB_B),
            "cda": cda,
            "cs": cs,
        }
        if has_b:
            bc = (b * np.float32(2.0**WS)).astype(np.float16)
            bd = ((b - bc.astype(np.float32) * np.float32(2.0**-WS)) * np.float32(2.0**WS)).astype(np.float16)
            z = np.zeros(E, np.float16)
            m["bcd"] = np.concatenate([bc, z, bd, z]).reshape(1, 4 * E)
        in_maps.append(m)

    nc = _build(has_b)
    res = run_bass_kernel_spmd(
        nc, in_maps, core_ids=list(range(NUM_CORES)), trace=TRACE
    )
    LAST_RESULTS = res

    wts = np.concatenate([r["wts"] for r in res.results], axis=1)
    gated = np.concatenate([r["gated"] for r in res.results], axis=1)
    return (
        gated.reshape(E, Bb, S).astype(np.float32),
        wts.reshape(E, Bb, S).astype(np.float32),
    )
